# revision 1
# baseline (speedup 1.0000x reference)
"""AnyVariateAttention Trainium2 kernel (8 NeuronCores, SPMD).

Sharding: 16 (batch, head) pairs / 8 cores -> each core computes 2 adjacent
heads of one batch (core c: batch c//4, heads 2*(c%4), 2*(c%4)+1).
Per core: QKV projection (transposed layouts), partial RoPE, flash-style
attention with transposed scores (S^T tiles [k,q]), block-bias folded into the
exp via the ACT bias port, softmax denominator via a ones-column appended to V,
normalization via reciprocal + ones-rank-1 broadcast matmul, and a partial
output projection. Host sums the 4 per-batch partials and transposes.
"""

import sys
import os
import numpy as np

for _p in ("/opt/trn_rl_repo",):
    if _p not in sys.path:
        sys.path.insert(0, _p)

import ml_dtypes

BF16 = ml_dtypes.bfloat16

B, N, D, H, HD = 2, 4096, 256, 8, 32
SEQ = 512
SCALE = HD ** -0.5
NCORES = 8
SCHRAUD_A = 184.6650390625  # 128 * log2(e)
SCHRAUD_B = 16256.0 - 7.4
# 116 of 256 exp tiles on DVE via bit-trick exp (well-mixed hash)

_NC_CACHE = {}


def _build_nc(stage=4):
    import concourse.bass as bass
    import concourse.tile as tile
    from concourse import bacc, mybir
    from concourse.bass import ts

    from concourse.alu_op_type import AluOpType
    bf = mybir.dt.bfloat16
    f32 = mybir.dt.float32
    i16 = mybir.dt.int16
    EXP = mybir.ActivationFunctionType.Exp

    nc = bacc.Bacc("TRN2", target_bir_lowering=False, debug=False, num_devices=NCORES)

    hsT_d = nc.declare_dram_parameter("hsT", [D, N], bf, isOutput=False)
    wq_d = nc.declare_dram_parameter("wq", [D, 64], bf, isOutput=False)
    wk_d = nc.declare_dram_parameter("wk", [D, 64], bf, isOutput=False)
    wv_d = nc.declare_dram_parameter("wv", [D, 64], bf, isOutput=False)
    wo_d = nc.declare_dram_parameter("wo", [32, 2 * D], bf, isOutput=False)
    cos_d = nc.declare_dram_parameter("rope_cos", [128, N], bf, isOutput=False)
    sin_d = nc.declare_dram_parameter("rope_sin", [128, N], bf, isOutput=False)
    bias_d = nc.declare_dram_parameter("biases", [128, 4], f32, isOutput=False)
    out_d = nc.declare_dram_parameter("outT", [D, N], f32, isOutput=True)

    NT = N // 512  # 8 q-tiles of 512
    NCP = N // 256  # 16 chunk-pairs (2x128 k rows each)

    with tile.TileContext(nc) as tc:
        from contextlib import ExitStack

        with ExitStack() as ctx:
            const = ctx.enter_context(tc.tile_pool(name="const", bufs=1))

            hs_sb = const.tile([128, 2, N], bf, tag="hs_sb")
            wq_sb = const.tile([128, 2, 64], bf, tag="wq_sb")
            wk_sb = const.tile([128, 2, 64], bf, tag="wk_sb")
            wv_sb = const.tile([128, 2, 64], bf, tag="wv_sb")
            wo_sb = const.tile([32, 2, D], bf, tag="wo_sb")
            cos_sb = const.tile([128, N], bf, tag="cos_sb")
            sin_sb = const.tile([128, N], bf, tag="sin_sb")
            bias_sb = const.tile([128, 4], f32, tag="bias_sb")
            schraud_sb = const.tile([128, 4], f32, tag="schraud_sb")
            ones_sb = const.tile([1, 32], f32, tag="ones_sb")
            qk_sb = const.tile([128, N], bf, tag="qk_sb")
            tmp_sb = const.tile([128, N], bf, tag="tmp_sb")
            Qd = const.tile([128, N], bf, tag="Qd")
            Kd = const.tile([128, N], bf, tag="Kd")
            # v tiles: [k-chunk 128, chunk, head, 35] with [v(32) | ones(1) | pad(2)]
            v_sb = const.tile([128, 32, 2, 35], bf, tag="v_sb")

            # --- input DMAs ---
            for d in range(2):
                nc.sync.dma_start(wq_sb[:, d, :], wq_d[ts(d, 128), :])
                nc.sync.dma_start(hs_sb[:, d, 0:1024],
                                  hsT_d[ts(d, 128), 0:1024])
                nc.sync.dma_start(wk_sb[:, d, :], wk_d[ts(d, 128), :])
                for cb in range(1, 4):
                    nc.sync.dma_start(hs_sb[:, d, ts(cb, 1024)],
                                      hsT_d[ts(d, 128), ts(cb, 1024)])
                nc.sync.dma_start(wv_sb[:, d, :], wv_d[ts(d, 128), :])
            nc.sync.dma_start(
                wo_sb[:], wo_d[:].rearrange("p (h j) -> p h j", h=2))
            nc.sync.dma_start(bias_sb[:], bias_d[:])
            for cb in range(4):
                nc.sync.dma_start(cos_sb[:, ts(cb, 1024)],
                                  cos_d[:, ts(cb, 1024)])
                nc.sync.dma_start(sin_sb[:, ts(cb, 1024)],
                                  sin_d[:, ts(cb, 1024)])
            nc.vector.tensor_scalar(
                schraud_sb[:], bias_sb[:], SCHRAUD_A, SCHRAUD_B,
                AluOpType.mult, AluOpType.add)
            nc.vector.memset(ones_sb[:], 1.0)
            nc.vector.memset(v_sb[:, :, :, 32:33], 1.0)

            # --- phase 1: q/k projections (transposed layout) + v (natural) ---
            with tc.tile_pool(name="qkp", bufs=2, space="PSUM") as qkp:
                for t in range(NT):
                    ps = qkp.tile([128, 512], f32, tag="qkps")
                    for d in range(2):
                        nc.tensor.matmul(
                            ps[0:64, :], lhsT=wq_sb[:, d, :],
                            rhs=hs_sb[:, d, ts(t, 512)],
                            start=(d == 0), stop=(d == 1), tile_position=(0, 0))
                        nc.tensor.matmul(
                            ps[64:128, :], lhsT=wk_sb[:, d, :],
                            rhs=hs_sb[:, d, ts(t, 512)],
                            start=(d == 0), stop=(d == 1), tile_position=(0, 64))
                    nc.scalar.copy(qk_sb[:, ts(t, 512)], ps[:])
            if stage == 1:
                ob = out_d[:].bitcast(bf)
                nc.sync.dma_start(ob[0:128, 0:N], qk_sb[:])
                nc.sync.dma_start(
                    ob[128 : 128 + 128, 0 : 32 * 2 * 35],
                    v_sb[:].rearrange("p a b c -> p (a b c)"))

            # --- phase 2: partial RoPE on q and k (rows: qA qB kA kB) ---
            if stage >= 2:
                for cb in range(4):
                    cs = ts(cb, 1024)
                    for g in (0, 2, 1, 3):
                        b0 = 32 * g
                        nc.sync.dma_start(
                            tmp_sb[b0:b0 + 16, cs], qk_sb[b0 + 16:b0 + 32, cs])
                        nc.sync.dma_start(
                            tmp_sb[b0 + 16:b0 + 32, cs], qk_sb[b0:b0 + 16, cs])
                    nc.vector.tensor_mul(tmp_sb[:, cs], tmp_sb[:, cs], sin_sb[:, cs])
                    nc.vector.tensor_mul(qk_sb[:, cs], qk_sb[:, cs], cos_sb[:, cs])
                    nc.vector.tensor_add(qk_sb[:, cs], qk_sb[:, cs], tmp_sb[:, cs])
                    # duplicate q and k to both halves (4-way row tiling)
                    nc.sync.dma_start(Qd[0:64, cs], qk_sb[0:64, cs])
                    nc.sync.dma_start(Qd[64:128, cs], qk_sb[0:64, cs])
                    nc.sync.dma_start(Kd[0:64, cs], qk_sb[64:128, cs])
                    nc.sync.dma_start(Kd[64:128, cs], qk_sb[64:128, cs])

            # v projection after rope emission so DVE ropes immediately
            with tc.tile_pool(name="vpp", bufs=2, space="PSUM") as vpp:
                for tv in range(32):
                    vp = vpp.tile([128, 64], f32, tag="vps")
                    for d in range(2):
                        nc.tensor.matmul(
                            vp[:], lhsT=hs_sb[:, d, ts(tv, 128)],
                            rhs=wv_sb[:, d, :],
                            start=(d == 0), stop=(d == 1))
                    nc.scalar.copy(
                        v_sb[:, tv, :, 0:32],
                        vp[:].rearrange("p (h x) -> p h x", h=2))

            if stage == 2:
                ob = out_d[:].bitcast(bf)
                nc.sync.dma_start(ob[0:128, 0:N], Qd[:])
                nc.sync.dma_start(ob[128:256, 0:N], Kd[:])

            # --- phase 3: attention main loop (software-pipelined
            # emission: PV trails scores by one step, the PE parts of the
            # norm/oproj chain trail by another, so the in-order PE queue
            # never sits behind an exp or reciprocal dependency) ---
            n_t = 0 if stage < 3 else (1 if stage == 3 else NT)
            with tc.tile_pool(name="spp", bufs=3, space="PSUM") as spp, \
                 tc.tile_pool(name="pvp", bufs=2, space="PSUM") as pvp, \
                 tc.tile_pool(name="ptp", bufs=7) as ptp, \
                 tc.tile_pool(name="mgp", bufs=3) as mgp, \
                 tc.tile_pool(name="rcp", bufs=4) as rcp, \
                 tc.tile_pool(name="ntp", bufs=3) as ntp:
                pv_tiles = {}

                def emit_scores_exp(t, cp):
                    sp = [spp.tile([128, 1024], f32, tag="sp",
                                   name=f"sp{t}_{cp}_{h}") for h in range(2)]
                    for g in (0, 2, 1, 3):
                        c = 2 * cp + (g // 2)
                        h = g % 2
                        half = g // 2
                        nc.tensor.matmul(
                            sp[h][:, ts(half, 512)],
                            lhsT=Kd[ts(g, 32), ts(c, 128)],
                            rhs=Qd[ts(g, 32), ts(t, 512)],
                            start=True, stop=True,
                            tile_position=(32 * g, 0))
                    pts = [None, None]
                    same = (cp // 2) == t
                    for h in range(2):
                        col = 2 * h + (0 if same else 1)
                        idx = (t * NCP + cp) * 2 + h
                        on_dve = ((idx * 21) % 50 < 23) and idx not in (62, 191)
                        if on_dve:
                            pt = ptp.tile([128, 1024], i16, tag="pt",
                                          name=f"pti{t}_{cp}_{h}")
                            nc.vector.tensor_scalar(
                                pt[:], sp[h][:], SCHRAUD_A,
                                schraud_sb[:, col:col + 1],
                                AluOpType.mult, AluOpType.add)
                            pts[h] = pt[:].bitcast(bf)
                        else:
                            pt = ptp.tile([128, 1024], bf, tag="pt",
                                          name=f"pt{t}_{cp}_{h}")
                            nc.scalar.activation(
                                pt[:], sp[h][:], EXP,
                                bias=bias_sb[:, col:col + 1], scale=1.0)
                            pts[h] = pt[:]
                    return pts

                def emit_pv(t, cp, pts):
                    if cp == 0:
                        pv_tiles[t] = [
                            pvp.tile([128, 512], f32, tag="pv",
                                     name=f"pv{t}_{h}") for h in range(2)]
                    pv = pv_tiles[t]
                    for h in range(2):
                        for j in range(2):
                            c = 2 * cp + j
                            pbase = 0 if j == 0 else 64
                            nc.tensor.matmul(
                                pv[h][pbase:pbase + 33, :],
                                lhsT=v_sb[:, c, h, 0:33],
                                rhs=pts[h][:, ts(j, 512)],
                                start=(cp == 0), stop=(cp == NCP - 1),
                                tile_position=(0, pbase))

                def emit_front(t):
                    pv = pv_tiles[t]
                    st = []
                    for h in range(2):
                        stage0 = mgp.tile([33, 512], f32, tag="stage0",
                                          name=f"stage0_{t}_{h}")
                        nc.scalar.copy(stage0[:], pv[h][64:97, :])
                        merged = mgp.tile([33, 512], f32, tag="mg",
                                          name=f"mg{t}_{h}")
                        nc.vector.tensor_add(
                            merged[:], pv[h][0:33, :], stage0[:])
                        den0 = rcp.tile([1, 512], f32, tag="den0",
                                        name=f"den0_{t}_{h}")
                        nc.sync.dma_start(den0[:], merged[32:33, :])
                        recip = rcp.tile([1, 512], f32, tag="rc",
                                         name=f"rc{t}_{h}")
                        nc.vector.reciprocal_approx_fast(
                            out=recip[:], in_=den0[:])
                        st.append((merged, recip))
                    return (t, st)

                def emit_tail(tail):
                    t, st = tail
                    numts = []
                    for h in range(2):
                        merged, recip = st[h]
                        bc = pvp.tile([32, 512], f32, tag="pv",
                                      name=f"bc{t}_{h}")
                        nc.tensor.matmul(
                            bc[:], lhsT=ones_sb[:], rhs=recip[:],
                            start=True, stop=True)
                        numt = ntp.tile([32, 512], bf, tag=f"numt{h}",
                                        name=f"numt{t}_{h}")
                        nc.vector.tensor_mul(numt[:], merged[0:32, :], bc[:])
                        numts.append(numt)
                    for jc in range(2):
                        op_ps = pvp.tile([128, 512], f32, tag="pv",
                                         name=f"op{t}_{jc}")
                        for h in range(2):
                            nc.tensor.matmul(
                                op_ps[:], lhsT=wo_sb[:, h, ts(jc, 128)],
                                rhs=numts[h][:],
                                start=(h == 0), stop=(h == 1))
                        op_sb = ntp.tile([128, 512], f32, tag="opsb",
                                         name=f"opsb{t}_{jc}")
                        nc.scalar.copy(op_sb[:], op_ps[:])
                        nc.sync.dma_start(
                            out_d[ts(jc, 128), ts(t, 512)], op_sb[:])

                from collections import deque
                pend_pv = deque()  # (t, cp, pts), emitted 2 steps later
                prev_front = None  # t
                prev_tail = None   # (t, st)
                for t in range(n_t):
                    for cp in range(NCP):
                        pts = emit_scores_exp(t, cp)
                        if prev_tail is not None:
                            emit_tail(prev_tail)
                            prev_tail = None
                        if len(pend_pv) >= 1:
                            pt_, pc_, pp_ = pend_pv.popleft()
                            emit_pv(pt_, pc_, pp_)
                            if pc_ == NCP - 1:
                                prev_front = pt_
                        if prev_front is not None:
                            prev_tail = emit_front(prev_front)
                            prev_front = None
                        pend_pv.append((t, cp, pts))
                while pend_pv:
                    pt_, pc_, pp_ = pend_pv.popleft()
                    emit_pv(pt_, pc_, pp_)
                    if prev_tail is not None:
                        emit_tail(prev_tail)
                        prev_tail = None
                    if pc_ == NCP - 1:
                        prev_front = pt_
                    if prev_front is not None:
                        prev_tail = emit_front(prev_front)
                        prev_front = None
                if prev_tail is not None:
                    emit_tail(prev_tail)
    nc.compile()
    return nc


def _rope_tables():
    j = np.arange(8, dtype=np.float64)
    inv = 10000.0 ** (-(2.0 * j / HD))  # [8]
    ang = np.arange(N, dtype=np.float64)[None, :] * inv[:, None]  # [8, N]
    cosb = np.ones((32, N), dtype=np.float64)
    sinb = np.zeros((32, N), dtype=np.float64)
    cosb[0:8] = np.cos(ang)
    cosb[16:24] = np.cos(ang)
    sinb[0:8] = -np.sin(ang)
    sinb[16:24] = np.sin(ang)
    cos128 = np.tile(cosb, (4, 1)).astype(BF16)
    sin128 = np.tile(sinb, (4, 1)).astype(BF16)
    return cos128, sin128


def kernel(**inputs):
    hs = np.asarray(inputs["hidden_states"], dtype=np.float32)
    qw = np.asarray(inputs["q_w"], dtype=np.float32)
    kw = np.asarray(inputs["k_w"], dtype=np.float32)
    vw = np.asarray(inputs["v_w"], dtype=np.float32)
    ow = np.asarray(inputs["o_w"], dtype=np.float32)
    ob = np.asarray(inputs["o_b"], dtype=np.float32)
    qb = np.asarray(inputs["q_b"], dtype=np.float32)
    kb = np.asarray(inputs["k_b"], dtype=np.float32)
    vb = np.asarray(inputs["v_b"], dtype=np.float32)
    ab = np.asarray(inputs["attention_biases"], dtype=np.float32)
    seq = int(np.asarray(inputs["sequence_length"]))
    assert seq == SEQ, f"kernel compiled for sequence_length={SEQ}, got {seq}"
    assert hs.shape == (B, N, D)
    assert not (np.any(qb) or np.any(kb) or np.any(vb)), "nonzero qkv bias unsupported"

    stage = int(os.environ.get("KERNEL_STAGE", "4"))
    if ("nc", stage) not in _NC_CACHE:
        _NC_CACHE[("nc", stage)] = _build_nc(stage)
    nc = _NC_CACHE[("nc", stage)]

    cos128, sin128 = _rope_tables()
    in_maps = []
    for c in range(NCORES):
        b = c // 4
        h0 = 2 * (c % 4)
        rows = slice(h0 * HD, h0 * HD + 2 * HD)
        in_maps.append({
            "hsT": np.ascontiguousarray(hs[b].T).astype(BF16),
            "wq": np.ascontiguousarray((qw[rows, :] * SCALE).T).astype(BF16),
            "wk": np.ascontiguousarray(kw[rows, :].T).astype(BF16),
            "wv": np.ascontiguousarray(vw[rows, :].T).astype(BF16),
            "wo": np.ascontiguousarray(
                ow[:, rows].T.reshape(2, 32, D).transpose(1, 0, 2)
                .reshape(32, 2 * D)).astype(BF16),
            "rope_cos": cos128,
            "rope_sin": sin128,
            "biases": np.ascontiguousarray(
                np.broadcast_to(ab[h0:h0 + 2].reshape(1, 4), (128, 4))
            ).astype(np.float32),
        })

    global _LAST_IN_MAPS, _LAST_RESULTS
    _LAST_IN_MAPS = in_maps
    from concourse.bass_utils import run_bass_kernel_spmd
    res = run_bass_kernel_spmd(nc, in_maps, core_ids=list(range(NCORES)))
    _LAST_RESULTS = res.results
    out = np.zeros((B, N, D), dtype=np.float32)
    for c in range(NCORES):
        out[c // 4] += res.results[c]["outT"].T.astype(np.float32)
    out += ob[None, None, :]
    return out

